# revision 32
# baseline (speedup 1.0000x reference)
"""Causal self-attention (B=2, T=2048, C=768, H=12) on 8 TRN2 NeuronCores.

Sharding: data-parallel over B (cores 0-3 -> b=0, cores 4-7 -> b=1), tensor
parallel over heads (3 heads per core). Each core computes q/k/v projections
for its 3 heads, causal attention, and a partial output projection; the host
sums the 4 partials per batch element and adds the output/v biases.

Attention is computed transposed: S^T[tk, tq] = K Q^T so that the softmax
denominator comes out of the ones-augmented AV matmul (V | 1) as row 64 of
the [65, 512] PSUM accumulator; exp runs on the scalar engine straight out
of PSUM with 1/sqrt(d) folded into the activation scale. Heads A and B are
packed onto PE row-strips 0-63 / 64-127 so their K=64 S-matmuls execute
concurrently. Diagonal tiles only compute their causally-valid column range.

The attention phase is ACT(exp)-bound, so the q/k/v projection matmuls for
chunk j+1 and the out-projection for chunk j-1 are injected between the
attention tile-steps of chunk j to keep PE busy while ACT drains.

The v bias never touches the device: softmax rows sum to 1, so its
contribution is the constant vector out_w @ qkv_b[2C:], added on the host.
"""

import numpy as np
import ml_dtypes
from contextlib import ExitStack

import concourse.bass as bass
import concourse.tile as tile
from concourse import bacc, mybir
from concourse.bass_utils import run_bass_kernel_spmd

BF16 = mybir.dt.bfloat16
F32 = mybir.dt.float32
AF = mybir.ActivationFunctionType

B, T, C, H, D = 2, 2048, 768, 12, 64
HPC = 3          # heads per core
NCORES = 8
CC = C // 128    # 6 contraction chunks
NT = T // 128    # 16 t tiles
NJ = T // 512    # 4 tq chunks
VW = D + 1       # 65: v columns + ones column
SCALE = float(D) ** -0.5

_cache = {}


def _build_program():
    nc = bacc.Bacc("TRN2", target_bir_lowering=False, debug=False,
                   enable_asserts=False, num_devices=NCORES)

    xt_d = nc.dram_tensor("xt_s", [128, CC * T], BF16, kind="ExternalInput").ap()
    wqk_d = nc.dram_tensor("wqk_s", [128, CC * 384], BF16, kind="ExternalInput").ap()
    wv_d = nc.dram_tensor("wv_s", [128, CC * 192], BF16, kind="ExternalInput").ap()
    bqk_d = nc.dram_tensor("bqk_s", [128, 3], F32, kind="ExternalInput").ap()
    w2ab_d = nc.dram_tensor("w2ab_s", [128, C], BF16, kind="ExternalInput").ap()
    # w2c rows duplicated to partitions 64-127 so w2c matmuls for adjacent
    # jt chunks can run concurrently on the two K=64 PE row strips
    w2c_d = nc.dram_tensor("w2c_s", [128, C], BF16, kind="ExternalInput").ap()
    masks_d = nc.dram_tensor("masks_s", [128, 4 * 512], BF16, kind="ExternalInput").ap()
    outp_d = nc.dram_tensor("outp", [128, CC * T], F32, kind="ExternalOutput").ap()

    with tile.TileContext(nc) as tc, ExitStack() as ctx:
        const = ctx.enter_context(tc.tile_pool(name="const", bufs=1))
        big = ctx.enter_context(tc.tile_pool(name="big", bufs=1))
        psum = ctx.enter_context(tc.tile_pool(name="psum", bufs=4, space="PSUM"))
        psum_pr = ctx.enter_context(tc.tile_pool(name="psum_pr", bufs=2, space="PSUM"))
        psum_av = ctx.enter_context(tc.tile_pool(name="psum_av", bufs=2, space="PSUM"))
        ppool = ctx.enter_context(tc.tile_pool(name="ppool", bufs=8))
        small = ctx.enter_context(tc.tile_pool(name="small", bufs=6))

        # warm the ACT exp table while DMAs are in flight
        warm = small.tile([1, 16], F32, tag="warm")
        nc.vector.memset(warm[:], 0.0)
        warm2 = small.tile([1, 16], F32, tag="warm")
        nc.scalar.activation(warm2[:], warm[:], AF.Exp)

        # ---- load constants/inputs (order matters: earliest consumers first)
        # wqk split per contraction chunk so the first qk matmul only waits
        # for its own 98KB slice (the qk group accumulates chunk-by-chunk
        # as the interleaved xt slices land)
        wqk = const.tile([128, CC * 384], BF16)
        bqk = const.tile([128, 3], F32)
        xt = const.tile([128, CC * T], BF16)
        nc.sync.dma_start(bqk[:], bqk_d[:])
        for kc in range(CC):
            nc.sync.dma_start(wqk[:, kc * 384:(kc + 1) * 384],
                              wqk_d[:, kc * 384:(kc + 1) * 384])
            sl = slice(kc * T, kc * T + 512)
            nc.sync.dma_start(xt[:, sl], xt_d[:, sl])
        wv = const.tile([128, CC * 192], BF16)
        nc.sync.dma_start(wv[:], wv_d[:])
        masks = const.tile([128, 4 * 512], BF16)
        nc.sync.dma_start(masks[:], masks_d[:])
        for kc in range(CC):
            sl = slice(kc * T + 512, kc * T + 1024)
            nc.sync.dma_start(xt[:, sl], xt_d[:, sl])
        for kc in range(CC):
            sl = slice(kc * T + 1024, kc * T + 2048)
            nc.sync.dma_start(xt[:, sl], xt_d[:, sl])
        w2ab = const.tile([128, C], BF16)
        nc.sync.dma_start(w2ab[:], w2ab_d[:])
        w2c = const.tile([128, C], BF16)
        nc.sync.dma_start(w2c[:], w2c_d[:])

        # ---- persistent intermediates
        qt1 = big.tile([128, T], BF16)   # qA (p 0-63) | qB (p 64-127), [d, t]
        kt1 = big.tile([128, T], BF16)   # kA | kB
        qkt2 = big.tile([128, T], BF16)  # qC | kC
        kt2 = big.tile([128, T], BF16)   # kC shifted to p 0-63 | qC copy at p 64-127
        vbuf = big.tile([128, NT * HPC * VW], BF16)  # per t-chunk: [vA 1|vB 1|vC 1]
        ot_ab = big.tile([128, T], BF16)  # O.T heads A,B (out-proj rhs chunk 0)
        ot_c = big.tile([128, T], BF16)   # O.T head C, duplicated on p 64-127

        # wz memset first: the warm-up matmuls gate on it, and vbuf isn't
        # read until the first v-copy ~12us in
        wz = const.tile([128, 512], BF16, name="wz")
        nc.vector.memset(wz[:], 0.0)
        nc.vector.memset(vbuf[:], 1.0)

        def s_operands(h, i, j, c0):
            """(lhsT=k-tile, rhs=q-chunk) for head h, tk-tile i, tq-chunk j.
            Head C alternates PE row strips by tile parity so adjacent
            tiles' K=64 S-matmuls execute concurrently (the second of a
            lo/hi pair is free): even tiles use kC/qC on partitions 0-63,
            odd tiles use the original kC rows 64-127 of qkt2 with the qC
            copy at partitions 64-127 of kt2."""
            it = slice(i * 128, (i + 1) * 128)
            qs = slice(j * 512 + c0, (j + 1) * 512)
            if h == 0:
                return kt1[0:64, it], qt1[0:64, qs]
            if h == 1:
                return kt1[64:128, it], qt1[64:128, qs]
            if i % 2 == 0:
                return kt2[0:64, it], qkt2[0:64, qs]
            return qkt2[64:128, it], kt2[64:128, qs]

        def v_ap(h, i):
            off = i * HPC * VW + h * VW
            return vbuf[:, off:off + VW]

        qk_dest = [qt1, kt1, qkt2]

        def emit_qk_group(jt, j):
            ps = psum_pr.tile([128, 512], F32, tag="proj", name=f"qk_{jt}_{j}")
            for kc in range(CC):
                nc.tensor.matmul(
                    ps[:],
                    wqk[:, kc * 384 + jt * 128: kc * 384 + (jt + 1) * 128],
                    xt[:, kc * T + j * 512: kc * T + (j + 1) * 512],
                    start=(kc == 0), stop=(kc == CC - 1),
                )
            nc.vector.tensor_scalar_add(
                qk_dest[jt][:, j * 512:(j + 1) * 512], ps[:], bqk[:, jt:jt + 1])

        def emit_shift(j):
            js_ = slice(j * 512, (j + 1) * 512)
            nc.sync.dma_start(kt2[0:64, js_], qkt2[64:128, js_])
            nc.sync.dma_start(kt2[64:128, js_], qkt2[0:64, js_])

        def emit_v_group(ti):
            ps = psum_pr.tile([128, 192], F32, tag="proj", name=f"v_{ti}")
            for kc in range(CC):
                nc.tensor.matmul(
                    ps[:],
                    xt[:, kc * T + ti * 128: kc * T + (ti + 1) * 128],
                    wv[:, kc * 192:(kc + 1) * 192],
                    start=(kc == 0), stop=(kc == CC - 1),
                )
            dst = vbuf[:, ti * HPC * VW:(ti + 1) * HPC * VW]
            dst = dst.rearrange("p (h x) -> p h x", h=HPC)[:, :, 0:D]
            nc.vector.tensor_copy(
                dst, ps[:].rearrange("p (h x) -> p h x", h=HPC))

        def emit_outproj_pair(jt0, j, strip_pair=True):
            """Out-projection for chunks jt0, jt0+1: the two K=128 w2ab
            matmuls first, then the two K=64 w2c matmuls on opposite PE
            row strips so they execute concurrently.  strip_pair=False
            keeps both w2c matmuls on the lo strip (used for the last
            chunk, whose ot_c hi-copy DMA would sit on the critical tail)."""
            js = slice(j * 512, (j + 1) * 512)
            ps = {}
            for m, jt in enumerate((jt0, jt0 + 1)):
                ps[jt] = psum_pr.tile([128, 512], F32, tag="proj",
                                      name=f"op_{jt}_{j}")
                nc.tensor.matmul(ps[jt][:], w2ab[:, jt * 128:(jt + 1) * 128],
                                 ot_ab[:, js], start=True, stop=False,
                                 skip_group_check=True)
            for m, jt in enumerate((jt0, jt0 + 1)):
                lo, hi = (64 * m, 64 * m + 64) if strip_pair else (0, 64)
                nc.tensor.matmul(ps[jt][:], w2c[lo:hi, jt * 128:(jt + 1) * 128],
                                 ot_c[lo:hi, js], start=False, stop=True,
                                 skip_group_check=True)
            for jt in (jt0, jt0 + 1):
                ob = small.tile([128, 512], F32, tag="ob", name=f"ob_{jt}_{j}")
                if strip_pair:
                    nc.vector.tensor_copy(ob[:], ps[jt][:])
                else:
                    # epilogue: DVE is busy with the last normalize chain,
                    # ACT is idle — route the PSUM drain there
                    nc.scalar.activation(ob[:], ps[jt][:], AF.Copy)
                nc.sync.dma_start(
                    outp_d[:, jt * T + j * 512: jt * T + (j + 1) * 512], ob[:])

        # warm-up matmuls: PE activity during the input DMA wait so the HAM
        # clock-gate reaches K=8/8 before real work starts
        wps = psum.tile([128, 512], F32, tag="s", name="wps")
        for _ in range(10):
            nc.tensor.matmul(wps[:], wz[:, 0:128], wz[:], start=True, stop=True)

        # prologue: just enough projections to start attention chunk 0
        emit_qk_group(0, 0)
        emit_qk_group(1, 0)
        emit_qk_group(2, 0)
        emit_shift(0)
        emit_v_group(0)
        emit_v_group(1)

        for j in range(NJ):
            js = slice(j * 512, (j + 1) * 512)
            n_i = 4 * j + 4

            # Work to inject between attention batches of this chunk.  The
            # exp load grows with j ((4j+4) tiles/head) while S+AV PE work
            # grows slower, so late chunks are ACT-paced with PE capacity to
            # spare: out-projections are deferred to chunks 2 and 3 instead
            # of running in the (PE-paced) chunk right after their own.
            inj = []
            if j == 0:
                inj += [lambda: emit_v_group(2), lambda: emit_v_group(3)]
            if j + 1 < NJ:
                inj += [lambda jt=jt: emit_qk_group(jt, j + 1) for jt in range(3)]
                inj.append(lambda: emit_shift(j + 1))
                inj += [lambda ti=ti: emit_v_group(ti)
                        for ti in range(4 * (j + 1), 4 * (j + 1) + 4)]
            if j == 2:
                inj += [lambda jt=jt: emit_outproj_pair(jt, 0)
                        for jt in (0, 2, 4)]
            if j == 3:
                inj += [lambda jt=jt, jp=jp: emit_outproj_pair(jt, jp)
                        for jp in (1, 2) for jt in (0, 2, 4)]

            def emit_dummy():
                # keeps the HAM clock-gate at K=8/8 through sparse stretches
                dps = psum.tile([128, 512], F32, tag="s", name="dps")
                for _ in range(2):
                    nc.tensor.matmul(dps[:], wz[:, 0:128], wz[:],
                                     start=True, stop=True)

            if j == NJ - 1:
                inj += [emit_dummy for _ in range(8)]
            # one inject slot per batch: group01 runs 2j+2 pair-batches,
            # head C runs j+1 quad-batches
            total_steps = 3 * j + 3
            step = 0

            def maybe_inject():
                nonlocal step
                step += 1
                remaining_slots = total_steps - step + 1
                k = -(-len(inj) // max(1, remaining_slots))  # ceil
                for _ in range(min(k, len(inj))):
                    inj.pop(0)()

            # tile list for this chunk: full tiles then diagonal tiles
            tiles = [(i, 0, False) for i in range(4 * j)] + \
                    [(4 * j + oi, 128 * oi, True) for oi in range(4)]

            # Tile-steps are processed in batches (2 steps for the A/B head
            # pair, 4 for head C) with all S matmuls of a batch first, then
            # the exps, then the PREVIOUS batch's AV matmuls.  Same-type
            # matmuls chain cleanly on the PE; each type transition costs
            # ~100ns (weight-load serialization), so batching halves/quarters
            # that tax.  The one-batch AV lag keeps AV matmuls from waiting
            # on a just-finished exp.
            for group, bs in (((0, 1), 2), ((2,), 4)):
                av = {h: psum_av.tile([VW, 512], F32, tag="av",
                                      name=f"av_{h}_{j}") for h in group}
                started = {h: False for h in group}
                pending = []  # batches of (h, i, pt_ap, col0)

                def flush_gen(last=False):
                    gen = pending.pop(0)
                    last_idx = {}
                    for idx, (h, _i, _pt, _c0) in enumerate(gen):
                        last_idx[h] = idx
                    for idx, (h, i, pt_ap, c0) in enumerate(gen):
                        nc.tensor.matmul(
                            av[h][:, c0:512], v_ap(h, i), pt_ap,
                            start=(not started[h]),
                            stop=(last and idx == last_idx[h]),
                            skip_group_check=True,
                        )
                        started[h] = True

                for b0 in range(0, len(tiles), bs):
                    batch = tiles[b0:b0 + bs]
                    sp = {}
                    for (i, c0, _diag) in batch:
                        for h in group:
                            t = psum.tile([128, 512 - c0], F32, tag="s",
                                          name=f"sp_{h}_{j}_{i}")
                            sp[(i, h)] = t
                            lhsT, rhs = s_operands(h, i, j, c0)
                            nc.tensor.matmul(t[:], lhsT, rhs,
                                             start=True, stop=True)
                    gen = []
                    for (i, c0, diag) in batch:
                        for h in group:
                            pt = ppool.tile([128, 512 - c0], BF16, tag="pt",
                                            name=f"pt_{h}_{j}_{i}")
                            nc.scalar.activation(pt[:], sp[(i, h)][:], AF.Exp,
                                                 scale=SCALE)
                            if diag:
                                oi = c0 // 128
                                nc.vector.tensor_mul(
                                    pt[:], pt[:],
                                    masks[:, oi * 512 + c0:(oi + 1) * 512])
                            gen.append((h, i, pt[:], c0))
                    pending.append(gen)
                    if len(pending) >= 2:
                        flush_gen()
                    maybe_inject()
                while len(pending) > 1:
                    flush_gen()
                if pending:
                    flush_gen(last=True)

                # normalize: O.T[d, tq] = av[0:64] / av[64]
                for h in group:
                    recip = small.tile([1, 512], F32, tag="recip",
                                       name=f"recip_{h}_{j}")
                    den = small.tile([1, 512], F32, tag="den",
                                     name=f"den_{h}_{j}")
                    nc.vector.tensor_copy(den[:], av[h][D:VW, :])
                    # custom-DVE ops read garbage from PSUM; SBUF source only
                    nc.vector.reciprocal_approx_fast(recip[:], den[:])
                    rb = small.tile([64, 512], F32, tag="rb", name=f"rb_{h}_{j}")
                    nc.gpsimd.partition_broadcast(rb[:], recip[:])
                    if h == 0:
                        dst = ot_ab[0:64, js]
                    elif h == 2:
                        dst = ot_c[0:64, js]
                    else:
                        dst = small.tile([64, 512], BF16, tag="otb",
                                         name=f"otb_{j}")
                    nc.vector.tensor_mul(dst[:], av[h][0:D, :], rb[:])
                    if h == 1:
                        nc.sync.dma_start(ot_ab[64:128, js], dst[:])
                    if h == 2 and j + 1 < NJ:
                        nc.sync.dma_start(ot_c[64:128, js], dst[:])

            # leftover injections for this chunk
            while inj:
                inj.pop(0)()

        # epilogue: out-projection for the last chunk, with PE kept warm
        # (the HAM clock-gate decays within ~1us of low matmul pressure and
        # epilogue matmuls then run at less than half speed)
        for jt0 in (0, 2, 4):
            emit_outproj_pair(jt0, NJ - 1, strip_pair=False)
            dps = psum.tile([128, 512], F32, tag="s", name=f"dps_e{jt0}")
            for _ in range(3):
                nc.tensor.matmul(dps[:], wz[:, 0:128], wz[:],
                                 start=True, stop=True)

    nc.compile()
    return nc


def _prep_in_maps(x, qkv_w, qkv_b, out_w):
    bf = ml_dtypes.bfloat16
    in_maps = []

    # causal masks for the 4 diagonal offsets: keep when f >= oi*128 + p
    p = np.arange(128)[:, None]
    f = np.arange(512)[None, :]
    masks = np.stack([(f >= oi * 128 + p) for oi in range(4)])  # [4,128,512]
    masks_s = np.ascontiguousarray(
        masks.transpose(1, 0, 2).reshape(128, 4 * 512)).astype(bf)

    for c in range(NCORES):
        b = c // 4
        h0 = (c % 4) * HPC
        hs = [h0, h0 + 1, h0 + 2]

        xT = np.ascontiguousarray(x[b].T.astype(np.float32))  # [768, 2048]
        xt_s = xT.reshape(CC, 128, T).transpose(1, 0, 2).reshape(128, CC * T)

        qr = lambda h: qkv_w[h * D:(h + 1) * D]
        kr = lambda h: qkv_w[C + h * D: C + (h + 1) * D]
        vr = lambda h: qkv_w[2 * C + h * D: 2 * C + (h + 1) * D]
        qb = lambda h: qkv_b[h * D:(h + 1) * D]
        kb = lambda h: qkv_b[C + h * D: C + (h + 1) * D]

        wqk = np.concatenate([qr(hs[0]), qr(hs[1]), kr(hs[0]), kr(hs[1]),
                              qr(hs[2]), kr(hs[2])], axis=0)  # [384, 768]
        wqk_s = np.ascontiguousarray(wqk.T).reshape(CC, 128, 384) \
            .transpose(1, 0, 2).reshape(128, CC * 384)
        wv_ = np.concatenate([vr(h) for h in hs], axis=0)      # [192, 768]
        wv_s = np.ascontiguousarray(wv_.T).reshape(CC, 128, 192) \
            .transpose(1, 0, 2).reshape(128, CC * 192)

        bqk = np.concatenate([qb(hs[0]), qb(hs[1]), kb(hs[0]), kb(hs[1]),
                              qb(hs[2]), kb(hs[2])])
        bqk_s = np.ascontiguousarray(bqk.reshape(3, 128).T).astype(np.float32)

        ch_ab = np.r_[hs[0] * D:(hs[0] + 1) * D, hs[1] * D:(hs[1] + 1) * D]
        ch_c = np.r_[hs[2] * D:(hs[2] + 1) * D]
        w2ab_s = np.ascontiguousarray(out_w[:, ch_ab].T)  # [128, 768]
        w2c_half = out_w[:, ch_c].T                       # [64, 768]
        w2c_s = np.ascontiguousarray(
            np.concatenate([w2c_half, w2c_half], axis=0))  # [128, 768]

        in_maps.append({
            "xt_s": np.ascontiguousarray(xt_s).astype(bf),
            "wqk_s": np.ascontiguousarray(wqk_s).astype(bf),
            "wv_s": np.ascontiguousarray(wv_s).astype(bf),
            "bqk_s": bqk_s,
            "w2ab_s": w2ab_s.astype(bf),
            "w2c_s": w2c_s.astype(bf),
            "masks_s": masks_s,
        })
    return in_maps


def _assemble(results, qkv_b, out_w, out_b):
    out = np.zeros((B, T, C), dtype=np.float32)
    for c in range(NCORES):
        b = c // 4
        outp = results[c]["outp"]  # [128, CC*T] f32
        outT = outp.reshape(128, CC, T).transpose(1, 0, 2).reshape(C, T)
        out[b] += outT.T
    # v-bias contribution (softmax rows sum to 1) + output bias
    const = out_w.astype(np.float32) @ qkv_b[2 * C:].astype(np.float32) \
        + out_b.astype(np.float32)
    out += const[None, None, :]
    return out


def run(x, qkv_w, qkv_b, out_w, out_b, trace=False, tmpdir=None):
    if "nc" not in _cache:
        _cache["nc"] = _build_program()
    nc = _cache["nc"]
    x = np.asarray(x, dtype=np.float32)
    qkv_w = np.asarray(qkv_w, dtype=np.float32)
    qkv_b = np.asarray(qkv_b, dtype=np.float32)
    out_w = np.asarray(out_w, dtype=np.float32)
    out_b = np.asarray(out_b, dtype=np.float32)
    in_maps = _prep_in_maps(x, qkv_w, qkv_b, out_w)
    res = run_bass_kernel_spmd(nc, in_maps, list(range(NCORES)), trace=trace,
                               tmpdir=tmpdir)
    out = _assemble(res.results, qkv_b, out_w, out_b)
    return out, res


def kernel(x, qkv_w, qkv_b, out_w, out_b):
    out, _ = run(x, qkv_w, qkv_b, out_w, out_b, trace=False)
    return out



# revision 37
# speedup vs baseline: 1.0194x; 1.0194x over previous
"""Causal self-attention (B=2, T=2048, C=768, H=12) on 8 TRN2 NeuronCores.

Sharding: data-parallel over B (cores 0-3 -> b=0, cores 4-7 -> b=1), tensor
parallel over heads (3 heads per core). Each core computes q/k/v projections
for its 3 heads, causal attention, and a partial output projection; the host
sums the 4 partials per batch element and adds the output/v biases.

Attention is computed transposed: S^T[tk, tq] = K Q^T so that the softmax
denominator comes out of the ones-augmented AV matmul (V | 1) as row 64 of
the [65, 512] PSUM accumulator; exp runs on the scalar engine straight out
of PSUM with 1/sqrt(d) folded into the activation scale. Heads A and B are
packed onto PE row-strips 0-63 / 64-127 so their K=64 S-matmuls execute
concurrently. Diagonal tiles only compute their causally-valid column range.

The attention phase is ACT(exp)-bound, so the q/k/v projection matmuls for
chunk j+1 and the out-projection for chunk j-1 are injected between the
attention tile-steps of chunk j to keep PE busy while ACT drains.

The v bias never touches the device: softmax rows sum to 1, so its
contribution is the constant vector out_w @ qkv_b[2C:], added on the host.
"""

import numpy as np
import ml_dtypes
from contextlib import ExitStack

import concourse.bass as bass
import concourse.tile as tile
from concourse import bacc, mybir
from concourse.bass_utils import run_bass_kernel_spmd

BF16 = mybir.dt.bfloat16
F32 = mybir.dt.float32
AF = mybir.ActivationFunctionType

B, T, C, H, D = 2, 2048, 768, 12, 64
HPC = 3          # heads per core
NCORES = 8
CC = C // 128    # 6 contraction chunks
NT = T // 128    # 16 t tiles
NJ = T // 512    # 4 tq chunks
VW = D + 1       # 65: v columns + ones column
SCALE = float(D) ** -0.5

_cache = {}


def _build_program():
    nc = bacc.Bacc("TRN2", target_bir_lowering=False, debug=False,
                   enable_asserts=False, num_devices=NCORES)

    xt_d = nc.dram_tensor("xt_s", [128, CC * T], BF16, kind="ExternalInput").ap()
    wqk_d = nc.dram_tensor("wqk_s", [128, CC * 384], BF16, kind="ExternalInput").ap()
    wv_d = nc.dram_tensor("wv_s", [128, CC * 192], BF16, kind="ExternalInput").ap()
    bqk_d = nc.dram_tensor("bqk_s", [128, 3], F32, kind="ExternalInput").ap()
    w2ab_d = nc.dram_tensor("w2ab_s", [128, C], BF16, kind="ExternalInput").ap()
    # w2c rows duplicated to partitions 64-127 so w2c matmuls for adjacent
    # jt chunks can run concurrently on the two K=64 PE row strips
    w2c_d = nc.dram_tensor("w2c_s", [128, C], BF16, kind="ExternalInput").ap()
    masks_d = nc.dram_tensor("masks_s", [128, 4 * 512], BF16, kind="ExternalInput").ap()
    outp_d = nc.dram_tensor("outp", [128, CC * T], F32, kind="ExternalOutput").ap()

    with tile.TileContext(nc) as tc, ExitStack() as ctx:
        const = ctx.enter_context(tc.tile_pool(name="const", bufs=1))
        big = ctx.enter_context(tc.tile_pool(name="big", bufs=1))
        # S psum: two 2-bank [128,1024] tiles; each holds a PAIR of S tiles
        # so one exp instruction covers both (ACT per-instr overhead ~185ns)
        psum = ctx.enter_context(tc.tile_pool(name="psum", bufs=2, space="PSUM"))
        psum_pr = ctx.enter_context(tc.tile_pool(name="psum_pr", bufs=2, space="PSUM"))
        psum_av = ctx.enter_context(tc.tile_pool(name="psum_av", bufs=2, space="PSUM"))
        ppool = ctx.enter_context(tc.tile_pool(name="ppool", bufs=8))
        small = ctx.enter_context(tc.tile_pool(name="small", bufs=6))

        # warm the ACT exp table while DMAs are in flight
        warm = small.tile([1, 16], F32, tag="warm")
        nc.vector.memset(warm[:], 0.0)
        warm2 = small.tile([1, 16], F32, tag="warm")
        nc.scalar.activation(warm2[:], warm[:], AF.Exp)

        # ---- load constants/inputs (order matters: earliest consumers first)
        # wqk split per contraction chunk so the first qk matmul only waits
        # for its own 98KB slice (the qk group accumulates chunk-by-chunk
        # as the interleaved xt slices land)
        wqk = const.tile([128, CC * 384], BF16)
        bqk = const.tile([128, 3], F32)
        xt = const.tile([128, CC * T], BF16)
        nc.sync.dma_start(bqk[:], bqk_d[:])
        for kc in range(CC):
            nc.sync.dma_start(wqk[:, kc * 384:(kc + 1) * 384],
                              wqk_d[:, kc * 384:(kc + 1) * 384])
            sl = slice(kc * T, kc * T + 512)
            nc.sync.dma_start(xt[:, sl], xt_d[:, sl])
        wv = const.tile([128, CC * 192], BF16)
        nc.sync.dma_start(wv[:], wv_d[:])
        masks = const.tile([128, 4 * 512], BF16)
        nc.sync.dma_start(masks[:], masks_d[:])
        for kc in range(CC):
            sl = slice(kc * T + 512, kc * T + 1024)
            nc.sync.dma_start(xt[:, sl], xt_d[:, sl])
        for kc in range(CC):
            sl = slice(kc * T + 1024, kc * T + 2048)
            nc.sync.dma_start(xt[:, sl], xt_d[:, sl])
        w2ab = const.tile([128, C], BF16)
        nc.sync.dma_start(w2ab[:], w2ab_d[:])
        w2c = const.tile([128, C], BF16)
        nc.sync.dma_start(w2c[:], w2c_d[:])

        # ---- persistent intermediates
        qt1 = big.tile([128, T], BF16)   # qA (p 0-63) | qB (p 64-127), [d, t]
        kt1 = big.tile([128, T], BF16)   # kA | kB
        qkt2 = big.tile([128, T], BF16)  # qC | kC
        kt2 = big.tile([128, T], BF16)   # kC shifted to p 0-63 | qC copy at p 64-127
        vbuf = big.tile([128, NT * HPC * VW], BF16)  # per t-chunk: [vA 1|vB 1|vC 1]
        ot_ab = big.tile([128, T], BF16)  # O.T heads A,B (out-proj rhs chunk 0)
        ot_c = big.tile([128, T], BF16)   # O.T head C, duplicated on p 64-127

        # wz memset first: the warm-up matmuls gate on it, and vbuf isn't
        # read until the first v-copy ~12us in
        wz = const.tile([128, 512], BF16, name="wz")
        nc.vector.memset(wz[:], 0.0)
        nc.vector.memset(vbuf[:], 1.0)

        def s_operands(h, i, j, c0):
            """(lhsT=k-tile, rhs=q-chunk) for head h, tk-tile i, tq-chunk j.
            Head C alternates PE row strips by tile parity so adjacent
            tiles' K=64 S-matmuls execute concurrently (the second of a
            lo/hi pair is free): even tiles use kC/qC on partitions 0-63,
            odd tiles use the original kC rows 64-127 of qkt2 with the qC
            copy at partitions 64-127 of kt2."""
            it = slice(i * 128, (i + 1) * 128)
            qs = slice(j * 512 + c0, (j + 1) * 512)
            if h == 0:
                return kt1[0:64, it], qt1[0:64, qs]
            if h == 1:
                return kt1[64:128, it], qt1[64:128, qs]
            if i % 2 == 0:
                return kt2[0:64, it], qkt2[0:64, qs]
            return qkt2[64:128, it], kt2[64:128, qs]

        def v_ap(h, i):
            off = i * HPC * VW + h * VW
            return vbuf[:, off:off + VW]

        qk_dest = [qt1, kt1, qkt2]

        def emit_qk_group(jt, j):
            ps = psum_pr.tile([128, 512], F32, tag="proj", name=f"qk_{jt}_{j}")
            for kc in range(CC):
                nc.tensor.matmul(
                    ps[:],
                    wqk[:, kc * 384 + jt * 128: kc * 384 + (jt + 1) * 128],
                    xt[:, kc * T + j * 512: kc * T + (j + 1) * 512],
                    start=(kc == 0), stop=(kc == CC - 1),
                )
            nc.vector.tensor_scalar_add(
                qk_dest[jt][:, j * 512:(j + 1) * 512], ps[:], bqk[:, jt:jt + 1])

        def emit_shift(j):
            js_ = slice(j * 512, (j + 1) * 512)
            nc.sync.dma_start(kt2[0:64, js_], qkt2[64:128, js_])
            nc.sync.dma_start(kt2[64:128, js_], qkt2[0:64, js_])

        def emit_v_group(ti):
            ps = psum_pr.tile([128, 192], F32, tag="proj", name=f"v_{ti}")
            for kc in range(CC):
                nc.tensor.matmul(
                    ps[:],
                    xt[:, kc * T + ti * 128: kc * T + (ti + 1) * 128],
                    wv[:, kc * 192:(kc + 1) * 192],
                    start=(kc == 0), stop=(kc == CC - 1),
                )
            dst = vbuf[:, ti * HPC * VW:(ti + 1) * HPC * VW]
            dst = dst.rearrange("p (h x) -> p h x", h=HPC)[:, :, 0:D]
            nc.vector.tensor_copy(
                dst, ps[:].rearrange("p (h x) -> p h x", h=HPC))

        def emit_outproj_pair(jt0, j, strip_pair=True):
            """Out-projection for chunks jt0, jt0+1: the two K=128 w2ab
            matmuls first, then the two K=64 w2c matmuls on opposite PE
            row strips so they execute concurrently.  strip_pair=False
            keeps both w2c matmuls on the lo strip (used for the last
            chunk, whose ot_c hi-copy DMA would sit on the critical tail)."""
            js = slice(j * 512, (j + 1) * 512)
            ps = {}
            for m, jt in enumerate((jt0, jt0 + 1)):
                ps[jt] = psum_pr.tile([128, 512], F32, tag="proj",
                                      name=f"op_{jt}_{j}")
                nc.tensor.matmul(ps[jt][:], w2ab[:, jt * 128:(jt + 1) * 128],
                                 ot_ab[:, js], start=True, stop=False,
                                 skip_group_check=True)
            for m, jt in enumerate((jt0, jt0 + 1)):
                lo, hi = (64 * m, 64 * m + 64) if strip_pair else (0, 64)
                nc.tensor.matmul(ps[jt][:], w2c[lo:hi, jt * 128:(jt + 1) * 128],
                                 ot_c[lo:hi, js], start=False, stop=True,
                                 skip_group_check=True)
            for jt in (jt0, jt0 + 1):
                ob = small.tile([128, 512], F32, tag="ob", name=f"ob_{jt}_{j}")
                if strip_pair:
                    nc.vector.tensor_copy(ob[:], ps[jt][:])
                else:
                    # epilogue: DVE is busy with the last normalize chain,
                    # ACT is idle — route the PSUM drain there
                    nc.scalar.activation(ob[:], ps[jt][:], AF.Copy)
                nc.sync.dma_start(
                    outp_d[:, jt * T + j * 512: jt * T + (j + 1) * 512], ob[:])

        # warm-up matmuls: PE activity during the input DMA wait so the HAM
        # clock-gate reaches K=8/8 before real work starts.  Warm-up and
        # dummy psum lives in the proj pool: its WAR waits (a proj copy from
        # two groups back) are long resolved by the time fillers run.
        wps = psum_pr.tile([128, 512], F32, tag="proj", name="wps")
        for _ in range(10):
            nc.tensor.matmul(wps[:], wz[:, 0:128], wz[:], start=True, stop=True)

        # prologue: just enough projections to start attention chunk 0
        emit_qk_group(0, 0)
        emit_qk_group(1, 0)
        emit_qk_group(2, 0)
        emit_shift(0)
        emit_v_group(0)
        emit_v_group(1)

        for j in range(NJ):
            js = slice(j * 512, (j + 1) * 512)
            n_i = 4 * j + 4

            # Work to inject between attention batches of this chunk.  The
            # exp load grows with j ((4j+4) tiles/head) while S+AV PE work
            # grows slower, so late chunks are ACT-paced with PE capacity to
            # spare: out-projections are deferred to chunks 2 and 3 instead
            # of running in the (PE-paced) chunk right after their own.
            inj = []
            if j == 0:
                inj += [lambda: emit_v_group(2), lambda: emit_v_group(3)]
            if j + 1 < NJ:
                inj += [lambda jt=jt: emit_qk_group(jt, j + 1) for jt in range(3)]
                inj.append(lambda: emit_shift(j + 1))
                inj += [lambda ti=ti: emit_v_group(ti)
                        for ti in range(4 * (j + 1), 4 * (j + 1) + 4)]
            if j == 2:
                inj += [lambda jt=jt: emit_outproj_pair(jt, 0)
                        for jt in (0, 2, 4)]
            if j == 3:
                inj += [lambda jt=jt, jp=jp: emit_outproj_pair(jt, jp)
                        for jp in (1, 2) for jt in (0, 2, 4)]

            def emit_dummy():
                # keeps the HAM clock-gate at K=8/8 through sparse stretches
                dps = psum_pr.tile([128, 512], F32, tag="proj", name="dps")
                for _ in range(2):
                    nc.tensor.matmul(dps[:], wz[:, 0:128], wz[:],
                                     start=True, stop=True)

            if j == NJ - 1:
                inj += [emit_dummy for _ in range(8)]
            # one inject slot per batch: group01 runs 2j+2 pair-batches,
            # head C runs j+1 quad-batches
            total_steps = 3 * j + 3
            step = 0

            def maybe_inject():
                nonlocal step
                step += 1
                remaining_slots = total_steps - step + 1
                k = -(-len(inj) // max(1, remaining_slots))  # ceil
                for _ in range(min(k, len(inj))):
                    inj.pop(0)()

            # tile list for this chunk: full tiles then diagonal tiles
            tiles = [(i, 0, False) for i in range(4 * j)] + \
                    [(4 * j + oi, 128 * oi, True) for oi in range(4)]

            # Tile-steps are processed in batches (2 steps for the A/B head
            # pair, 4 for head C) with all S matmuls of a batch first, then
            # the exps, then the PREVIOUS batch's AV matmuls.  Same-type
            # matmuls chain cleanly on the PE; each type transition costs
            # ~100ns (weight-load serialization), so batching halves/quarters
            # that tax.  The one-batch AV lag keeps AV matmuls from waiting
            # on a just-finished exp.
            for group, bs in (((0, 1), 2), ((2,), 4)):
                av = {h: psum_av.tile([VW, 512], F32, tag="av",
                                      name=f"av_{h}_{j}") for h in group}
                started = {h: False for h in group}
                pending = []  # batches of (h, i, pt_ap, col0)

                def flush_gen(last=False):
                    gen = pending.pop(0)
                    last_idx = {}
                    for idx, (h, _i, _pt, _c0) in enumerate(gen):
                        last_idx[h] = idx
                    for idx, (h, i, pt_ap, c0) in enumerate(gen):
                        nc.tensor.matmul(
                            av[h][:, c0:512], v_ap(h, i), pt_ap,
                            start=(not started[h]),
                            stop=(last and idx == last_idx[h]),
                            skip_group_check=True,
                        )
                        started[h] = True

                for b0 in range(0, len(tiles), bs):
                    batch = tiles[b0:b0 + bs]
                    # pack S tiles pairwise into [128,1024] psum tiles at
                    # column offsets 0/512 (bank-aligned): group01 pairs the
                    # two heads of one tile, head C pairs adjacent tiles
                    if len(group) == 2:
                        pairs = [((i, c0, diag, group[0], 0),
                                  (i, c0, diag, group[1], 512))
                                 for (i, c0, diag) in batch]
                    else:
                        h = group[0]
                        pairs = [((batch[k][0], batch[k][1], batch[k][2], h, 0),
                                  (batch[k + 1][0], batch[k + 1][1],
                                   batch[k + 1][2], h, 512))
                                 for k in range(0, len(batch), 2)]
                    sws = []
                    for pair in pairs:
                        sw = psum.tile([128, 1024], F32, tag="sw",
                                       name=f"sw_{j}_{pair[0][0]}_{pair[0][3]}")
                        for (i, c0, diag, h, off) in pair:
                            lhsT, rhs = s_operands(h, i, j, c0)
                            nc.tensor.matmul(sw[:, off:off + 512 - c0],
                                             lhsT, rhs, start=True, stop=True)
                        sws.append(sw)
                    gen = []
                    for sw, pair in zip(sws, pairs):
                        pt = ppool.tile([128, 1024], BF16, tag="pt",
                                        name=f"pt_{j}_{pair[0][0]}_{pair[0][3]}")
                        if pair[0][1] == pair[1][1]:
                            # equal widths: one exp over both halves via a
                            # [2, w] strided access pattern
                            w = 512 - pair[0][1]
                            src = sw[:].rearrange("p (b x) -> p b x", b=2)[:, :, 0:w]
                            dst = pt[:].rearrange("p (b x) -> p b x", b=2)[:, :, 0:w]
                            nc.scalar.activation(dst, src, AF.Exp, scale=SCALE)
                        else:
                            for (i, c0, diag, h, off) in pair:
                                w = 512 - c0
                                nc.scalar.activation(pt[:, off:off + w],
                                                     sw[:, off:off + w],
                                                     AF.Exp, scale=SCALE)
                        for (i, c0, diag, h, off) in pair:
                            w = 512 - c0
                            pslice = pt[:, off:off + w]
                            if diag:
                                oi = c0 // 128
                                nc.vector.tensor_mul(
                                    pslice, pslice,
                                    masks[:, oi * 512 + c0:(oi + 1) * 512])
                            gen.append((h, i, pslice, c0))
                    pending.append(gen)
                    if len(pending) >= 2:
                        flush_gen()
                    maybe_inject()
                while len(pending) > 1:
                    flush_gen()
                if pending:
                    flush_gen(last=True)

                # normalize: O.T[d, tq] = av[0:64] / av[64]
                for h in group:
                    recip = small.tile([1, 512], F32, tag="recip",
                                       name=f"recip_{h}_{j}")
                    den = small.tile([1, 512], F32, tag="den",
                                     name=f"den_{h}_{j}")
                    nc.vector.tensor_copy(den[:], av[h][D:VW, :])
                    # custom-DVE ops read garbage from PSUM; SBUF source only
                    nc.vector.reciprocal_approx_fast(recip[:], den[:])
                    rb = small.tile([64, 512], F32, tag="rb", name=f"rb_{h}_{j}")
                    nc.gpsimd.partition_broadcast(rb[:], recip[:])
                    if h == 0:
                        dst = ot_ab[0:64, js]
                    elif h == 2:
                        dst = ot_c[0:64, js]
                    else:
                        dst = small.tile([64, 512], BF16, tag="otb",
                                         name=f"otb_{j}")
                    nc.vector.tensor_mul(dst[:], av[h][0:D, :], rb[:])
                    if h == 1:
                        nc.sync.dma_start(ot_ab[64:128, js], dst[:])
                    if h == 2 and j + 1 < NJ:
                        nc.sync.dma_start(ot_c[64:128, js], dst[:])

            # leftover injections for this chunk
            while inj:
                inj.pop(0)()

        # epilogue: out-projection for the last chunk, with PE kept warm
        # (the HAM clock-gate decays within ~1us of low matmul pressure and
        # epilogue matmuls then run at less than half speed)
        for jt0 in (0, 2, 4):
            emit_outproj_pair(jt0, NJ - 1, strip_pair=False)
            dps = psum_pr.tile([128, 512], F32, tag="proj", name=f"dps_e{jt0}")
            for _ in range(3):
                nc.tensor.matmul(dps[:], wz[:, 0:128], wz[:],
                                 start=True, stop=True)

    nc.compile()
    return nc


def _prep_in_maps(x, qkv_w, qkv_b, out_w):
    bf = ml_dtypes.bfloat16
    in_maps = []

    # causal masks for the 4 diagonal offsets: keep when f >= oi*128 + p
    p = np.arange(128)[:, None]
    f = np.arange(512)[None, :]
    masks = np.stack([(f >= oi * 128 + p) for oi in range(4)])  # [4,128,512]
    masks_s = np.ascontiguousarray(
        masks.transpose(1, 0, 2).reshape(128, 4 * 512)).astype(bf)

    for c in range(NCORES):
        b = c // 4
        h0 = (c % 4) * HPC
        hs = [h0, h0 + 1, h0 + 2]

        xT = np.ascontiguousarray(x[b].T.astype(np.float32))  # [768, 2048]
        xt_s = xT.reshape(CC, 128, T).transpose(1, 0, 2).reshape(128, CC * T)

        qr = lambda h: qkv_w[h * D:(h + 1) * D]
        kr = lambda h: qkv_w[C + h * D: C + (h + 1) * D]
        vr = lambda h: qkv_w[2 * C + h * D: 2 * C + (h + 1) * D]
        qb = lambda h: qkv_b[h * D:(h + 1) * D]
        kb = lambda h: qkv_b[C + h * D: C + (h + 1) * D]

        wqk = np.concatenate([qr(hs[0]), qr(hs[1]), kr(hs[0]), kr(hs[1]),
                              qr(hs[2]), kr(hs[2])], axis=0)  # [384, 768]
        wqk_s = np.ascontiguousarray(wqk.T).reshape(CC, 128, 384) \
            .transpose(1, 0, 2).reshape(128, CC * 384)
        wv_ = np.concatenate([vr(h) for h in hs], axis=0)      # [192, 768]
        wv_s = np.ascontiguousarray(wv_.T).reshape(CC, 128, 192) \
            .transpose(1, 0, 2).reshape(128, CC * 192)

        bqk = np.concatenate([qb(hs[0]), qb(hs[1]), kb(hs[0]), kb(hs[1]),
                              qb(hs[2]), kb(hs[2])])
        bqk_s = np.ascontiguousarray(bqk.reshape(3, 128).T).astype(np.float32)

        ch_ab = np.r_[hs[0] * D:(hs[0] + 1) * D, hs[1] * D:(hs[1] + 1) * D]
        ch_c = np.r_[hs[2] * D:(hs[2] + 1) * D]
        w2ab_s = np.ascontiguousarray(out_w[:, ch_ab].T)  # [128, 768]
        w2c_half = out_w[:, ch_c].T                       # [64, 768]
        w2c_s = np.ascontiguousarray(
            np.concatenate([w2c_half, w2c_half], axis=0))  # [128, 768]

        in_maps.append({
            "xt_s": np.ascontiguousarray(xt_s).astype(bf),
            "wqk_s": np.ascontiguousarray(wqk_s).astype(bf),
            "wv_s": np.ascontiguousarray(wv_s).astype(bf),
            "bqk_s": bqk_s,
            "w2ab_s": w2ab_s.astype(bf),
            "w2c_s": w2c_s.astype(bf),
            "masks_s": masks_s,
        })
    return in_maps


def _assemble(results, qkv_b, out_w, out_b):
    out = np.zeros((B, T, C), dtype=np.float32)
    for c in range(NCORES):
        b = c // 4
        outp = results[c]["outp"]  # [128, CC*T] f32
        outT = outp.reshape(128, CC, T).transpose(1, 0, 2).reshape(C, T)
        out[b] += outT.T
    # v-bias contribution (softmax rows sum to 1) + output bias
    const = out_w.astype(np.float32) @ qkv_b[2 * C:].astype(np.float32) \
        + out_b.astype(np.float32)
    out += const[None, None, :]
    return out


def run(x, qkv_w, qkv_b, out_w, out_b, trace=False, tmpdir=None):
    if "nc" not in _cache:
        _cache["nc"] = _build_program()
    nc = _cache["nc"]
    x = np.asarray(x, dtype=np.float32)
    qkv_w = np.asarray(qkv_w, dtype=np.float32)
    qkv_b = np.asarray(qkv_b, dtype=np.float32)
    out_w = np.asarray(out_w, dtype=np.float32)
    out_b = np.asarray(out_b, dtype=np.float32)
    in_maps = _prep_in_maps(x, qkv_w, qkv_b, out_w)
    res = run_bass_kernel_spmd(nc, in_maps, list(range(NCORES)), trace=trace,
                               tmpdir=tmpdir)
    out = _assemble(res.results, qkv_b, out_w, out_b)
    return out, res


def kernel(x, qkv_w, qkv_b, out_w, out_b):
    out, _ = run(x, qkv_w, qkv_b, out_w, out_b, trace=False)
    return out

